# revision 24
# baseline (speedup 1.0000x reference)
"""Single-head attention kernel for Trainium2, SPMD over 8 NeuronCores.

Problem: x [4,4096,128], Wq/Wk/Wv [128,128] -> y [4,4096,128]
  q = x @ Wq.T ; k = x @ Wk.T ; v = x @ Wv.T
  y = softmax(q k^T / sqrt(128)) v

Sharding: 8 cores = 4 batches x 2 query-halves. Each core receives its
batch's x rotated so that its 2048 queries are rows 0..2047 (attention is
invariant to permuting the key order, so rotation changes nothing) -> all
cores run the identical NEFF with no dynamic offsets and no collectives.

Per-core dataflow (all attention matmuls bf16 inputs, f32 PSUM accum):
  xT chunks: load f32, cast on DVE, transpose on PE (bf16)
  M = Wq^T @ Wk (no transposes needed); uT = M^T @ xT[:2048]; v = x @ Wv^T
  per 1024-query block, per 32 key tiles:
      S^T = xT-tile^T @ uT-block     (PE, 2x N=512 into [128k,1024] PSUM)
      A^T = exp(S^T * scale)         (ACT, one op per 1024, bf16 SBUF)
      yT += v-tile^T @ A^T           (PE, [128o,1024q] PSUM accum)
      3-level bf16 pair-tree of A^T  (DVE+GpSimd, softmax denominator)
  l = ones^T @ tree-roots (PE), y = transpose(yT)*(1/l) (PE + DVE)

The emission is a single software-pipelined instruction stream: the PE
executes strictly in program order, so prep chunks 2..7 and each block's
epilogue are sliced into small pieces and interleaved into the next
block's kt loop, keeping the ACT exp stream (the throughput bound) fed
from ~10us after kernel entry until the end.
"""

import sys

sys.path.insert(0, "/opt/trn_rl_repo")

import numpy as np

import concourse.bass as bass
import concourse.mybir as mybir
from concourse import bacc
from concourse.bass_utils import run_bass_kernel_spmd
from concourse.tile import TileContext
from concourse.masks import make_identity

P = 128
N = 4096  # context length (per batch)
NQ = 2048  # queries per core
H = 128
O = 128
KT = N // P  # 32 key tiles
NC = N // 512  # 8 column chunks of 512
QBS = 1024  # query block size
QB = NQ // QBS  # 2 query blocks
SCALE = 1.0 / np.sqrt(128.0)
# Schraudolph exp-as-bf16-bits: bf16bits(exp(s*SCALE)) ~= s*ES0 + ES1
ES0 = float(128.0 * np.log2(np.e) * SCALE)
ES1 = float((127 << 7) - 8.0)

F32 = mybir.dt.float32
BF16 = mybir.dt.bfloat16

_cached_nc = None


def build_kernel():
    nc = bacc.Bacc(None, target_bir_lowering=False)

    x_d = nc.declare_dram_parameter("x", [N, H], F32, isOutput=False)
    w_d = {
        "q": nc.declare_dram_parameter("wq", [H, H], F32, isOutput=False),
        "k": nc.declare_dram_parameter("wk", [H, H], F32, isOutput=False),
        "v": nc.declare_dram_parameter("wv", [O, H], F32, isOutput=False),
    }
    out_d = nc.declare_dram_parameter("out", [NQ, O], F32, isOutput=True)

    with TileContext(nc) as tc:
        with (
            tc.tile_pool(name="const", bufs=1) as cpool,
            tc.tile_pool(name="big", bufs=1) as big,
            tc.tile_pool(name="stagea", bufs=4) as sta,
            tc.tile_pool(name="psum", bufs=2, space="PSUM") as psum,
            tc.tile_pool(name="apool", bufs=6) as apool,
            tc.tile_pool(name="tpool", bufs=3) as tpool,
            tc.tile_pool(name="epi", bufs=2) as epi,
        ):
            wq_n0 = None  # W DMAs issued first; identity ops follow on gpsimd

            xTs = [big.tile([P, 512], BF16, name=f"xT{c}") for c in range(NC)]
            uTs = [big.tile([P, 512], BF16, name=f"uT{c}") for c in range(NQ // 512)]
            xcs = [big.tile([P, 512], BF16, name=f"xc{c}") for c in range(NC)]
            wq_n = big.tile([P, P], BF16, name="wq_n")
            wk_n = big.tile([P, P], BF16, name="wk_n")
            wvT = big.tile([P, P], BF16, name="wvT")
            m_sb = big.tile([P, P], BF16, name="m_sb")

            def kslice(tiles, kt):
                return tiles[kt // 4][:, (kt % 4) * P : (kt % 4 + 1) * P]

            # ---------- prep piece emitters ----------
            def emit_w_setup_loads():
                # HWDGE f32 loads (fast completion) + DVE casts
                wf = sta.tile([P, 3 * P], F32, tag="wf")
                for wi, name in enumerate(("q", "k", "v")):
                    nc.sync.dma_start(
                        out=wf[:, wi * P : (wi + 1) * P], in_=w_d[name][:]
                    )
                for wi, wt in enumerate((wq_n, wk_n)):
                    nc.vector.tensor_copy(wt[:], wf[:, wi * P : (wi + 1) * P])
                wst = sta.tile([P, P], BF16, tag="wst")
                nc.vector.tensor_copy(wst[:], wf[:, 2 * P : 3 * P])
                ident_bf_ = cpool.tile([P, P], BF16, name="ident_bf")
                make_identity(nc, ident_bf_)
                ones_bf_ = cpool.tile([P, 1], BF16, name="ones_bf")
                nc.gpsimd.memset(ones_bf_[:], 1.0)
                return ident_bf_, ones_bf_, wst

            def emit_w_setup_pe(wst):
                pm = psum.tile([P, P], F32, tag="sm")
                nc.tensor.matmul(pm[:], wq_n[:], wk_n[:], start=True, stop=True)
                nc.vector.tensor_copy(m_sb[:], pm[:])
                pw = psum.tile([P, 2 * P], BF16, tag="sm")
                nc.tensor.transpose(pw[:, 0:P], wst[:], ident_bf[:])
                nc.vector.tensor_copy(wvT[:], pw[:, 0:P])

            chunk_state = {}

            def chunk_load(c):
                xst = sta.tile([P, 4, P], F32, tag="xst", name=f"xst{c}")
                rows = x_d[c * 512 : (c + 1) * 512, :]
                nc.sync.dma_start(
                    out=xst[:], in_=rows.rearrange("(t p) h -> p t h", p=P)
                )
                nc.vector.tensor_copy(
                    xcs[c][:], xst.rearrange("p t h -> p (t h)")
                )  # f32 -> bf16

            def chunk_transpose(c, half=None):
                if half in (None, 0):
                    px = psum.tile([P, 512], BF16, tag="sm", name=f"px{c}")
                    chunk_state[(c, "px")] = px
                px = chunk_state[(c, "px")]
                rng = range(4) if half is None else range(half * 2, half * 2 + 2)
                for t4 in rng:
                    nc.tensor.transpose(
                        px[:, t4 * P : (t4 + 1) * P],
                        xcs[c][:, t4 * P : (t4 + 1) * P],
                        ident_bf[:],
                    )
                if half in (None, 1):
                    nc.vector.tensor_copy(xTs[c][:], px[:])
                    del chunk_state[(c, "px")]

            def chunk_u(c):
                pu = psum.tile([P, 512], F32, tag="sm", name=f"pu{c}")
                nc.tensor.matmul(pu[:], m_sb[:], xTs[c][:], start=True, stop=True)
                nc.scalar.activation(
                    uTs[c][:], pu[:], mybir.ActivationFunctionType.Copy
                )

            # ---------- attention emitters ----------
            a_tiles = {}

            def emit_s_exp(qb, kt):
                ps = psum.tile([P, QBS], F32, tag="ps")
                for h in range(QBS // 512):
                    nc.tensor.matmul(
                        ps[:, h * 512 : (h + 1) * 512],
                        kslice(xTs, kt),
                        uTs[qb * 2 + h][:],
                        start=True, stop=True,
                    )
                a = apool.tile([P, QBS], BF16, tag="a")
                if kt % 4 == 2:
                    # Schraudolph: affine map + f32->int16 convert writes the
                    # bf16 bit pattern of exp(s*SCALE) (~2% elementwise, which
                    # the softmax renormalization cancels to ~0.1% on y)
                    nc.vector.tensor_scalar(
                        a.bitcast(mybir.dt.int16)[:], ps[:], ES0, ES1,
                        mybir.AluOpType.mult, mybir.AluOpType.add,
                    )
                else:
                    nc.scalar.activation(
                        a[:], ps[:], mybir.ActivationFunctionType.Exp,
                        scale=float(SCALE),
                    )
                a_tiles[(qb, kt)] = a

            def new_blk(qb):
                return {
                    "qb": qb,
                    "py": psum.tile([P, QBS], F32, tag="py", bufs=1,
                                    name=f"py{qb}"),
                    "tree": {1: None, 2: None},
                    "roots": [],
                }

            def emit_kt(b, kt):
                qb = b["qb"]
                a = a_tiles.pop((qb, kt))
                for h in range(QBS // 512):
                    nc.tensor.matmul(
                        b["py"][:, h * 512 : (h + 1) * 512],
                        kslice(xcs, kt),
                        a[:, h * 512 : (h + 1) * 512],
                        start=(kt == 0), stop=(kt == KT - 1),
                    )
                tree = b["tree"]
                if tree[1] is None:
                    tree[1] = a
                else:
                    p1 = tpool.tile([P, QBS], BF16, tag="t1")
                    eng = nc.gpsimd if (kt % 4 == 1 and kt < KT - 8) else nc.vector
                    eng.tensor_tensor(
                        p1[:], tree[1][:], a[:], mybir.AluOpType.add
                    )
                    tree[1] = None
                    if tree[2] is None:
                        tree[2] = p1
                    else:
                        p2 = tpool.tile([P, QBS], BF16, tag="t2")
                        nc.vector.tensor_tensor(
                            p2[:], tree[2][:], p1[:], mybir.AluOpType.add
                        )
                        tree[2] = None
                        b["roots"].append(p2)

            def epilogue_pieces(b, last=False):
                """The block epilogue as a list of small closures."""
                qb = b["qb"]
                st = {}

                def p_l0():
                    st["pls"] = [
                        psum.tile([1, 512], F32, tag="sm", name=f"pl{qb}_{h}")
                        for h in range(2)
                    ]
                    roots = b["roots"]
                    for ri, r in enumerate(roots):
                        for h in range(2):
                            nc.tensor.matmul(
                                st["pls"][h][:],
                                ones_bf[:],
                                r[:, h * 512 : (h + 1) * 512],
                                start=(ri == 0), stop=(ri == len(roots) - 1),
                            )

                def p_l1():
                    l_row = epi.tile([1, QBS], BF16, tag="l_row",
                                     name=f"lrow{qb}")
                    for h in range(2):
                        nc.scalar.activation(
                            l_row[:, h * 512 : (h + 1) * 512], st["pls"][h][:],
                            mybir.ActivationFunctionType.Copy,
                        )
                    st["l_row"] = l_row
                    w_sb = epi.tile([P, QBS], BF16, tag="w_sb", name=f"w{qb}")
                    for h in range(2):
                        nc.vector.tensor_copy(
                            w_sb[:, h * 512 : (h + 1) * 512],
                            b["py"][:, h * 512 : (h + 1) * 512],
                        )
                    st["w_sb"] = w_sb

                def p_j(g, j4):
                    j = g * 4 + j4
                    if j4 == 0:
                        st[f"yout{g}"] = epi.tile(
                            [P, 4, P], F32, tag="yout", name=f"yout{qb}_{g}"
                        )
                    psm_l = psum.tile([P, 1], BF16, tag="sm")
                    nc.tensor.transpose(
                        psm_l[:], st["l_row"][:, j * P : (j + 1) * P],
                        ident_bf[0:1, 0:1],
                    )
                    psm_y = psum.tile([P, P], F32, tag="ps" if last else "sm")
                    nc.tensor.matmul(
                        psm_y[:], st["w_sb"][:, j * P : (j + 1) * P], wvT[:],
                        start=True, stop=True,
                    )
                    lcol = epi.tile([P, 1], F32, tag="lcol")
                    nc.vector.reciprocal(lcol[:], psm_l[:])
                    nc.vector.tensor_scalar_mul(
                        st[f"yout{g}"][:, j4, :], psm_y[:], lcol[:, 0:1]
                    )
                    if j4 == 3:
                        r0 = qb * QBS + g * 512
                        nc.sync.dma_start(
                            out=out_d[r0 : r0 + 512, :].rearrange(
                                "(t p) h -> p t h", p=P
                            ),
                            in_=st[f"yout{g}"][:],
                        )

                pieces = [p_l0, p_l1]
                for g in range(2):
                    for j4 in range(4):
                        pieces.append(lambda g=g, j4=j4: p_j(g, j4))
                return pieces

            # ---------- emission schedule ----------
            DEPTH = 2
            ident_bf, ones_bf, _wst = emit_w_setup_loads()
            chunk_load(0)
            chunk_load(1)
            chunk_transpose(0)
            emit_w_setup_pe(_wst)
            chunk_u(0)
            chunk_transpose(1)
            chunk_u(1)
            blk = None
            for qb in range(QB):
                if qb == 0:
                    for kt in range(DEPTH):
                        emit_s_exp(0, kt)
                nxt = new_blk(qb)
                if blk is None:
                    # prep pieces for chunks 4..7, spread over kts
                    todo = []
                    for c in range(2, NC):
                        todo.append(lambda c=c: chunk_load(c))
                        todo.append(lambda c=c: chunk_transpose(c, 0))
                        todo.append(lambda c=c: chunk_transpose(c, 1))
                        if c < NQ // 512:
                            todo.append(lambda c=c: chunk_u(c))
                else:
                    todo = epilogue_pieces(blk)
                blk = nxt

                for kt in range(KT):
                    if kt + DEPTH < KT:
                        emit_s_exp(qb, kt + DEPTH)
                    elif qb + 1 < QB:
                        emit_s_exp(qb + 1, kt + DEPTH - KT)
                    emit_kt(blk, kt)
                    if todo:
                        todo.pop(0)()
                while todo:
                    todo.pop(0)()
            for piece in epilogue_pieces(blk, last=True):
                piece()

    nc.compile()
    return nc


def _run(x, Wq, Wk, Wv, **spmd_kwargs):
    global _cached_nc
    if _cached_nc is None:
        _cached_nc = build_kernel()
    nc = _cached_nc

    x = np.asarray(x, dtype=np.float32)
    Wq = np.ascontiguousarray(np.asarray(Wq, dtype=np.float32))
    Wk = np.ascontiguousarray(np.asarray(Wk, dtype=np.float32))
    Wv = np.ascontiguousarray(np.asarray(Wv, dtype=np.float32))

    B = x.shape[0]
    in_maps = []
    for core in range(8):
        b, half = core // 2, core % 2
        xb = x[b]
        if half:
            xb = np.roll(xb, -NQ, axis=0)  # queries -> rows 0..NQ-1
        in_maps.append(
            {"x": np.ascontiguousarray(xb), "wq": Wq, "wk": Wk, "wv": Wv}
        )

    res = run_bass_kernel_spmd(nc, in_maps, core_ids=list(range(8)), **spmd_kwargs)

    y = np.empty((B, N, O), dtype=np.float32)
    for core in range(8):
        b, half = core // 2, core % 2
        y[b, half * NQ : (half + 1) * NQ] = res.results[core]["out"]
    return y, res


def kernel(x, Wq, Wk, Wv):
    y, _ = _run(x, Wq, Wk, Wv)
    return y


if __name__ == "__main__":
    rng = np.random.default_rng(0)
    x = rng.standard_normal((4, N, H), dtype=np.float32)
    Wq = rng.standard_normal((H, H), dtype=np.float32) / np.sqrt(H)
    Wk = rng.standard_normal((H, H), dtype=np.float32) / np.sqrt(H)
    Wv = rng.standard_normal((O, H), dtype=np.float32) / np.sqrt(H)
    y = kernel(x=x, Wq=Wq, Wk=Wk, Wv=Wv)
    print("kernel output", y.shape, y.dtype)


# revision 25
# speedup vs baseline: 1.0077x; 1.0077x over previous
"""Single-head attention kernel for Trainium2, SPMD over 8 NeuronCores.

Problem: x [4,4096,128], Wq/Wk/Wv [128,128] -> y [4,4096,128]
  q = x @ Wq.T ; k = x @ Wk.T ; v = x @ Wv.T
  y = softmax(q k^T / sqrt(128)) v

Sharding: 8 cores = 4 batches x 2 query-halves. Each core receives its
batch's x rotated so that its 2048 queries are rows 0..2047 (attention is
invariant to permuting the key order, so rotation changes nothing) -> all
cores run the identical NEFF with no dynamic offsets and no collectives.

Per-core dataflow (all attention matmuls bf16 inputs, f32 PSUM accum):
  xT chunks: load f32, cast on DVE, transpose on PE (bf16)
  M = Wq^T @ Wk (no transposes needed); uT = M^T @ xT[:2048]; v = x @ Wv^T
  per 1024-query block, per 32 key tiles:
      S^T = xT-tile^T @ uT-block     (PE, 2x N=512 into [128k,1024] PSUM)
      A^T = exp(S^T * scale)         (ACT, one op per 1024, bf16 SBUF)
      yT += v-tile^T @ A^T           (PE, [128o,1024q] PSUM accum)
      3-level bf16 pair-tree of A^T  (DVE+GpSimd, softmax denominator)
  l = ones^T @ tree-roots (PE), y = transpose(yT)*(1/l) (PE + DVE)

The emission is a single software-pipelined instruction stream: the PE
executes strictly in program order, so prep chunks 2..7 and each block's
epilogue are sliced into small pieces and interleaved into the next
block's kt loop, keeping the ACT exp stream (the throughput bound) fed
from ~10us after kernel entry until the end.
"""

import sys

sys.path.insert(0, "/opt/trn_rl_repo")

import numpy as np

import concourse.bass as bass
import concourse.mybir as mybir
from concourse import bacc
from concourse.bass_utils import run_bass_kernel_spmd
from concourse.tile import TileContext
from concourse.masks import make_identity

P = 128
N = 4096  # context length (per batch)
NQ = 2048  # queries per core
H = 128
O = 128
KT = N // P  # 32 key tiles
NC = N // 512  # 8 column chunks of 512
QBS = 1024  # query block size
QB = NQ // QBS  # 2 query blocks
SCALE = 1.0 / np.sqrt(128.0)
# Schraudolph exp-as-bf16-bits: bf16bits(exp(s*SCALE)) ~= s*ES0 + ES1
ES0 = float(128.0 * np.log2(np.e) * SCALE)
ES1 = float((127 << 7) - 8.0)

F32 = mybir.dt.float32
BF16 = mybir.dt.bfloat16

_cached_nc = None


def build_kernel():
    nc = bacc.Bacc(None, target_bir_lowering=False)

    x_d = nc.declare_dram_parameter("x", [N, H], F32, isOutput=False)
    w_d = {
        "q": nc.declare_dram_parameter("wq", [H, H], F32, isOutput=False),
        "k": nc.declare_dram_parameter("wk", [H, H], F32, isOutput=False),
        "v": nc.declare_dram_parameter("wv", [O, H], F32, isOutput=False),
    }
    out_d = nc.declare_dram_parameter("out", [NQ, O], F32, isOutput=True)

    with TileContext(nc) as tc:
        with (
            tc.tile_pool(name="const", bufs=1) as cpool,
            tc.tile_pool(name="big", bufs=1) as big,
            tc.tile_pool(name="stagea", bufs=4) as sta,
            tc.tile_pool(name="psum", bufs=2, space="PSUM") as psum,
            tc.tile_pool(name="apool", bufs=6) as apool,
            tc.tile_pool(name="tpool", bufs=3) as tpool,
            tc.tile_pool(name="epi", bufs=2) as epi,
        ):
            wq_n0 = None  # W DMAs issued first; identity ops follow on gpsimd

            xTs = [big.tile([P, 512], BF16, name=f"xT{c}") for c in range(NC)]
            uTs = [big.tile([P, 512], BF16, name=f"uT{c}") for c in range(NQ // 512)]
            xcs = [big.tile([P, 512], BF16, name=f"xc{c}") for c in range(NC)]
            wq_n = big.tile([P, P], BF16, name="wq_n")
            wk_n = big.tile([P, P], BF16, name="wk_n")
            wvT = big.tile([P, P], BF16, name="wvT")
            m_sb = big.tile([P, P], BF16, name="m_sb")

            def kslice(tiles, kt):
                return tiles[kt // 4][:, (kt % 4) * P : (kt % 4 + 1) * P]

            # ---------- prep piece emitters ----------
            def emit_w_setup_loads():
                nc.gpsimd.dma_start(out=wq_n[:], in_=w_d["q"][:])  # cast
                nc.gpsimd.dma_start(out=wk_n[:], in_=w_d["k"][:])  # cast
                wst = sta.tile([P, P], BF16, tag="wst")
                nc.gpsimd.dma_start(out=wst[:], in_=w_d["v"][:])  # cast
                ident_bf_ = cpool.tile([P, P], BF16, name="ident_bf")
                make_identity(nc, ident_bf_)
                ones_bf_ = cpool.tile([P, 1], BF16, name="ones_bf")
                nc.gpsimd.memset(ones_bf_[:], 1.0)
                return ident_bf_, ones_bf_, wst

            def emit_w_setup_pe(wst):
                pm = psum.tile([P, P], F32, tag="sm")
                nc.tensor.matmul(pm[:], wq_n[:], wk_n[:], start=True, stop=True)
                nc.vector.tensor_copy(m_sb[:], pm[:])
                pw = psum.tile([P, 2 * P], BF16, tag="sm")
                nc.tensor.transpose(pw[:, 0:P], wst[:], ident_bf[:])
                nc.vector.tensor_copy(wvT[:], pw[:, 0:P])

            chunk_state = {}

            def chunk_load(c):
                xst = sta.tile([P, 4, P], F32, tag="xst", name=f"xst{c}")
                rows = x_d[c * 512 : (c + 1) * 512, :]
                nc.sync.dma_start(
                    out=xst[:], in_=rows.rearrange("(t p) h -> p t h", p=P)
                )
                nc.vector.tensor_copy(
                    xcs[c][:], xst.rearrange("p t h -> p (t h)")
                )  # f32 -> bf16

            def chunk_transpose(c, half=None):
                if half in (None, 0):
                    px = psum.tile([P, 512], BF16, tag="sm", name=f"px{c}")
                    chunk_state[(c, "px")] = px
                px = chunk_state[(c, "px")]
                rng = range(4) if half is None else range(half * 2, half * 2 + 2)
                for t4 in rng:
                    nc.tensor.transpose(
                        px[:, t4 * P : (t4 + 1) * P],
                        xcs[c][:, t4 * P : (t4 + 1) * P],
                        ident_bf[:],
                    )
                if half in (None, 1):
                    nc.vector.tensor_copy(xTs[c][:], px[:])
                    del chunk_state[(c, "px")]

            def chunk_u(c):
                pu = psum.tile([P, 512], F32, tag="sm", name=f"pu{c}")
                nc.tensor.matmul(pu[:], m_sb[:], xTs[c][:], start=True, stop=True)
                nc.scalar.activation(
                    uTs[c][:], pu[:], mybir.ActivationFunctionType.Copy
                )

            # ---------- attention emitters ----------
            a_tiles = {}

            def emit_s_exp(qb, kt):
                ps = psum.tile([P, QBS], F32, tag="ps")
                for h in range(QBS // 512):
                    nc.tensor.matmul(
                        ps[:, h * 512 : (h + 1) * 512],
                        kslice(xTs, kt),
                        uTs[qb * 2 + h][:],
                        start=True, stop=True,
                    )
                a = apool.tile([P, QBS], BF16, tag="a")
                if kt % 4 == 2:
                    # Schraudolph: affine map + f32->int16 convert writes the
                    # bf16 bit pattern of exp(s*SCALE) (~2% elementwise, which
                    # the softmax renormalization cancels to ~0.1% on y)
                    nc.vector.tensor_scalar(
                        a.bitcast(mybir.dt.int16)[:], ps[:], ES0, ES1,
                        mybir.AluOpType.mult, mybir.AluOpType.add,
                    )
                else:
                    nc.scalar.activation(
                        a[:], ps[:], mybir.ActivationFunctionType.Exp,
                        scale=float(SCALE),
                    )
                a_tiles[(qb, kt)] = a

            def new_blk(qb):
                return {
                    "qb": qb,
                    "py": psum.tile([P, QBS], F32, tag="py", bufs=1,
                                    name=f"py{qb}"),
                    "tree": {1: None, 2: None},
                    "roots": [],
                }

            def emit_kt(b, kt):
                qb = b["qb"]
                a = a_tiles.pop((qb, kt))
                for h in range(QBS // 512):
                    nc.tensor.matmul(
                        b["py"][:, h * 512 : (h + 1) * 512],
                        kslice(xcs, kt),
                        a[:, h * 512 : (h + 1) * 512],
                        start=(kt == 0), stop=(kt == KT - 1),
                    )
                tree = b["tree"]
                if tree[1] is None:
                    tree[1] = a
                else:
                    p1 = tpool.tile([P, QBS], BF16, tag="t1")
                    eng = nc.gpsimd if (kt % 4 == 1 and kt < KT - 8) else nc.vector
                    eng.tensor_tensor(
                        p1[:], tree[1][:], a[:], mybir.AluOpType.add
                    )
                    tree[1] = None
                    if tree[2] is None:
                        tree[2] = p1
                    else:
                        p2 = tpool.tile([P, QBS], BF16, tag="t2")
                        nc.vector.tensor_tensor(
                            p2[:], tree[2][:], p1[:], mybir.AluOpType.add
                        )
                        tree[2] = None
                        b["roots"].append(p2)

            def epilogue_pieces(b, last=False):
                """The block epilogue as a list of small closures."""
                qb = b["qb"]
                st = {}

                def p_l0():
                    st["pls"] = [
                        psum.tile([1, 512], F32, tag="sm", name=f"pl{qb}_{h}")
                        for h in range(2)
                    ]
                    roots = b["roots"]
                    for ri, r in enumerate(roots):
                        for h in range(2):
                            nc.tensor.matmul(
                                st["pls"][h][:],
                                ones_bf[:],
                                r[:, h * 512 : (h + 1) * 512],
                                start=(ri == 0), stop=(ri == len(roots) - 1),
                            )

                def p_l1():
                    l_row = epi.tile([1, QBS], BF16, tag="l_row",
                                     name=f"lrow{qb}")
                    for h in range(2):
                        nc.scalar.activation(
                            l_row[:, h * 512 : (h + 1) * 512], st["pls"][h][:],
                            mybir.ActivationFunctionType.Copy,
                        )
                    st["l_row"] = l_row
                    w_sb = epi.tile([P, QBS], BF16, tag="w_sb", name=f"w{qb}")
                    for h in range(2):
                        nc.vector.tensor_copy(
                            w_sb[:, h * 512 : (h + 1) * 512],
                            b["py"][:, h * 512 : (h + 1) * 512],
                        )
                    st["w_sb"] = w_sb

                def p_j(g, j4):
                    j = g * 4 + j4
                    if j4 == 0:
                        st[f"yout{g}"] = epi.tile(
                            [P, 4, P], F32, tag="yout", name=f"yout{qb}_{g}"
                        )
                    psm_l = psum.tile([P, 1], BF16, tag="sm")
                    nc.tensor.transpose(
                        psm_l[:], st["l_row"][:, j * P : (j + 1) * P],
                        ident_bf[0:1, 0:1],
                    )
                    psm_y = psum.tile([P, P], F32, tag="ps" if last else "sm")
                    nc.tensor.matmul(
                        psm_y[:], st["w_sb"][:, j * P : (j + 1) * P], wvT[:],
                        start=True, stop=True,
                    )
                    lcol = epi.tile([P, 1], F32, tag="lcol")
                    nc.vector.reciprocal(lcol[:], psm_l[:])
                    nc.vector.tensor_scalar_mul(
                        st[f"yout{g}"][:, j4, :], psm_y[:], lcol[:, 0:1]
                    )
                    if j4 == 3:
                        r0 = qb * QBS + g * 512
                        nc.sync.dma_start(
                            out=out_d[r0 : r0 + 512, :].rearrange(
                                "(t p) h -> p t h", p=P
                            ),
                            in_=st[f"yout{g}"][:],
                        )

                pieces = [p_l0, p_l1]
                for g in range(2):
                    for j4 in range(4):
                        pieces.append(lambda g=g, j4=j4: p_j(g, j4))
                return pieces

            # ---------- emission schedule ----------
            DEPTH = 2
            ident_bf, ones_bf, _wst = emit_w_setup_loads()
            chunk_load(0)
            chunk_load(1)
            chunk_transpose(0)
            emit_w_setup_pe(_wst)
            chunk_u(0)
            chunk_transpose(1)
            chunk_u(1)
            blk = None
            for qb in range(QB):
                if qb == 0:
                    for kt in range(DEPTH):
                        emit_s_exp(0, kt)
                nxt = new_blk(qb)
                if blk is None:
                    # prep pieces for chunks 4..7, spread over kts
                    todo = []
                    for c in range(2, NC):
                        todo.append(lambda c=c: chunk_load(c))
                        todo.append(lambda c=c: chunk_transpose(c, 0))
                        todo.append(lambda c=c: chunk_transpose(c, 1))
                        if c < NQ // 512:
                            todo.append(lambda c=c: chunk_u(c))
                else:
                    todo = epilogue_pieces(blk)
                blk = nxt

                for kt in range(KT):
                    if kt + DEPTH < KT:
                        emit_s_exp(qb, kt + DEPTH)
                    elif qb + 1 < QB:
                        emit_s_exp(qb + 1, kt + DEPTH - KT)
                    emit_kt(blk, kt)
                    if todo:
                        todo.pop(0)()
                while todo:
                    todo.pop(0)()
            for piece in epilogue_pieces(blk, last=True):
                piece()

    nc.compile()
    return nc


def _run(x, Wq, Wk, Wv, **spmd_kwargs):
    global _cached_nc
    if _cached_nc is None:
        _cached_nc = build_kernel()
    nc = _cached_nc

    x = np.asarray(x, dtype=np.float32)
    Wq = np.ascontiguousarray(np.asarray(Wq, dtype=np.float32))
    Wk = np.ascontiguousarray(np.asarray(Wk, dtype=np.float32))
    Wv = np.ascontiguousarray(np.asarray(Wv, dtype=np.float32))

    B = x.shape[0]
    in_maps = []
    for core in range(8):
        b, half = core // 2, core % 2
        xb = x[b]
        if half:
            xb = np.roll(xb, -NQ, axis=0)  # queries -> rows 0..NQ-1
        in_maps.append(
            {"x": np.ascontiguousarray(xb), "wq": Wq, "wk": Wk, "wv": Wv}
        )

    res = run_bass_kernel_spmd(nc, in_maps, core_ids=list(range(8)), **spmd_kwargs)

    y = np.empty((B, N, O), dtype=np.float32)
    for core in range(8):
        b, half = core // 2, core % 2
        y[b, half * NQ : (half + 1) * NQ] = res.results[core]["out"]
    return y, res


def kernel(x, Wq, Wk, Wv):
    y, _ = _run(x, Wq, Wk, Wv)
    return y


if __name__ == "__main__":
    rng = np.random.default_rng(0)
    x = rng.standard_normal((4, N, H), dtype=np.float32)
    Wq = rng.standard_normal((H, H), dtype=np.float32) / np.sqrt(H)
    Wk = rng.standard_normal((H, H), dtype=np.float32) / np.sqrt(H)
    Wv = rng.standard_normal((O, H), dtype=np.float32) / np.sqrt(H)
    y = kernel(x=x, Wq=Wq, Wk=Wk, Wv=Wv)
    print("kernel output", y.shape, y.dtype)
